# revision 34
# baseline (speedup 1.0000x reference)
"""Single-head attention (B=8, S=2048, D=1024, H=64) on 8 TRN2 NeuronCores.

Sharding: data-parallel over batch — one batch element per core, Q/K/V
weights replicated. No collectives; host gathers the 8 per-core outputs.

Host-side layout prep (per core): x fed pre-transposed as xT [D, S] bf16,
mask fed pre-transposed as maskT [S, S] bf16 (0.0/1.0), weights fed as
W^T bf16 with W_q/W_k fused into one [D, 128] stationary block.

Per-core pipeline:
  phase 1: qkT [128, S] = (Wqk^T)-stationary matmuls over 8 d-chunks
           (q rows 0-63, k rows 64-127), bias via tensor_scalar_add;
           kq_sb [128, S] = partition-swapped copy (SBUF->SBUF DMA) so
           kT also lives at partitions 0-63 and qT at 64-127;
           vT [64, S] similarly, then PE-transposed into v_aug [S,H+1]
           with a ones column (gives softmax denominators for free).
           Block-0 score pairs are interleaved into this loop so the
           scalar engine's exp chain starts as early as possible.
  phase 2: per 512-wide q-block: scoresT [k,q] computed directly
           (kT stationary, qT moving; K=64 row-tiled 2x: pair of k-tiles
           runs concurrently in the two PE row halves), exp via scalar
           ACT (scale=1/8), mask applied as a bf16 multiply;
           outT[65, q] += v_aug[k-tile].T @ probsT (PSUM accum over k)
           PE-transpose back, multiply by reciprocal of the ones-row,
           DMA out.  PV of block b-1 is emitted between score blocks to
           avoid head-of-line stalls on the PE queue.

Queue assignment: sync = x + mask input streams; gpsimd = weights, kq
partition-swap, output (pure DMA dispatch, no compute); scalar = exp (+
v_aug copies); vector = bias adds, mask multiplies, normalize.
"""

import sys
import types

import numpy as np
import ml_dtypes

import concourse.bass as bass
import concourse.mybir as mybir
import concourse.tile as tile
from concourse import bacc
from concourse.bass_utils import run_bass_kernel_spmd
from concourse.masks import make_identity

B, S, D, H = 8, 2048, 1024, 64
NT = S // 128          # 16 k-tiles of 128
NCH = D // 128         # 8 contraction chunks
NB = 4                 # q-blocks of 512

f32 = mybir.dt.float32
bf16 = mybir.dt.bfloat16
ACT_EXP = mybir.ActivationFunctionType.Exp


def install_ntff_hook():
    """RL-container antenv stub lacks axon_hooks; inject it so trace=True
    under axon can capture NTFF profiles. Harmless if already present."""
    if "antenv.axon_hooks" in sys.modules:
        return
    try:
        mod = types.ModuleType("antenv.axon_hooks")
        state = {"hook": None}
        mod.set_axon_ntff_profile_hook = lambda h: state.__setitem__("hook", h)
        mod.get_axon_ntff_profile_hook = lambda: state["hook"]
        sys.modules["antenv.axon_hooks"] = mod
        import antenv

        antenv.axon_hooks = mod
        from trn_agent_boot.trn_boot import _ntff_profile_via_ctypes

        mod.set_axon_ntff_profile_hook(
            _ntff_profile_via_ctypes("/opt/axon/libaxon_pjrt.so")
        )
    except Exception:
        pass


def build():
    nc = bacc.Bacc("TRN2", target_bir_lowering=False, debug=False, num_devices=8)

    xT_d = nc.dram_tensor("xT", [128, NCH * S], bf16, kind="ExternalInput")
    mT_d = nc.dram_tensor("maskT", [128, NT * S], bf16, kind="ExternalInput")
    wqk_d = nc.dram_tensor("wqk", [128, NCH * 128], bf16, kind="ExternalInput")
    wv_d = nc.dram_tensor("wv", [128, NCH * H], bf16, kind="ExternalInput")
    bqk_d = nc.dram_tensor("bqk", [128], f32, kind="ExternalInput")
    bv_d = nc.dram_tensor("bv", [H], f32, kind="ExternalInput")
    out_d = nc.dram_tensor("out", [S, H], f32, kind="ExternalOutput")

    with tile.TileContext(nc) as tc:
        with (
            tc.tile_pool(name="singles", bufs=1) as singles,
            tc.tile_pool(name="sb", bufs=2) as sb,
            tc.tile_pool(name="esb", bufs=6) as esb,
            tc.tile_pool(name="pP", bufs=2, space="PSUM") as pP,
            tc.tile_pool(name="pS", bufs=2, space="PSUM") as pS,
            tc.tile_pool(name="pM", bufs=1, space="PSUM") as pM,
            tc.tile_pool(name="pPV", bufs=1, space="PSUM") as pPV,
        ):
            # ---- constants -------------------------------------------------
            id_b = singles.tile([128, 128], bf16)
            make_identity(nc, id_b[:])
            id_f = singles.tile([128, 128], f32)
            make_identity(nc, id_f[:])
            id2_sb = singles.tile([128, 512], bf16)
            nc.gpsimd.memset(id2_sb[:], 1.0)

            def emit_warm(n=256):
                wps = pP.tile([128, 512], f32, tag="P", name="warm")
                nc.tensor.matmul(
                    wps[:, 0:n],
                    id_b[:],
                    id2_sb[:, 0:n],
                    start=True,
                    stop=True,
                )

            # HAM pre-burst: saturate the PE with no-dep fillers so the clock
            # gate opens before the first real matmul arrives.
            for _ in range(6):
                emit_warm(512)

            # ---- sync ring: x half 0, weights, x half 1, mask groups -------
            xT_tiles = [
                singles.tile([128, 2, S], bf16, name=f"xT{i}") for i in range(4)
            ]
            wqk_sb = singles.tile([128, NCH, 128], bf16)
            wv_sb = singles.tile([128, NCH, H], bf16)
            bqk_sb = singles.tile([128, 1], f32)
            bv_sb = singles.tile([H, 1], f32)
            m_bf = singles.tile([128, NT, S], bf16)

            def dma_x_q(qq):
                nc.sync.dma_start(
                    xT_tiles[qq][:],
                    bass.AP(
                        tensor=xT_d,
                        offset=qq * 2 * S,
                        ap=[[NCH * S, 128], [1, 2 * S]],
                    ),
                )

            nc.sync.dma_start(
                wqk_sb[:],
                bass.AP(tensor=wqk_d, offset=0, ap=[[NCH * 128, 128], [1, NCH * 128]]),
            )
            nc.sync.dma_start(
                wv_sb[:],
                bass.AP(tensor=wv_d, offset=0, ap=[[NCH * H, 128], [1, NCH * H]]),
            )
            nc.sync.dma_start(
                bqk_sb[:], bass.AP(tensor=bqk_d, offset=0, ap=[[1, 128], [0, 1]])
            )
            nc.sync.dma_start(
                bv_sb[:], bass.AP(tensor=bv_d, offset=0, ap=[[1, H], [0, 1]])
            )
            for qq in range(4):
                dma_x_q(qq)
            for g in range(4):
                nc.sync.dma_start(
                    m_bf[:, g * 4:(g + 1) * 4, :],
                    bass.AP(
                        tensor=mT_d,
                        offset=g * 4 * S,
                        ap=[[NT * S, 128], [1, 4 * S]],
                    ),
                )

            # ---- persistent activations -----------------------------------
            qkT_sb = singles.tile([128, S], bf16)   # q rows 0-63, k rows 64-127
            kq_sb = singles.tile([128, S], bf16)    # k rows 0-63, q rows 64-127
            vT_sb = singles.tile([H, S], bf16)
            v_aug = singles.tile([128, NT, H + 1], bf16)
            nc.gpsimd.memset(v_aug[:, :, H:H + 1], 1.0)

            probsT = singles.tile([128, NB, NT, 512], bf16)
            oT_tiles = [None] * NB

            def emit_proj_qk(blk):
                sl = slice(blk * 512, (blk + 1) * 512)
                qk_ps = pP.tile([128, 512], f32, tag="P")
                for c in range(NCH):
                    nc.tensor.matmul(
                        qk_ps[:],
                        wqk_sb[:, c, :],
                        xT_tiles[c // 2][:, c % 2, sl],
                        start=(c == 0),
                        stop=(c == NCH - 1),
                    )
                nc.vector.tensor_scalar_add(qkT_sb[:, sl], qk_ps[:], bqk_sb[:])
                # partition swap: kT to rows 0-63, qT duplicate to rows 64-127
                nc.gpsimd.dma_start(kq_sb[0:64, sl], qkT_sb[64:128, sl])
                nc.gpsimd.dma_start(kq_sb[64:128, sl], qkT_sb[0:64, sl])

            def emit_proj_v(blk):
                sl = slice(blk * 512, (blk + 1) * 512)
                vT_ps = pP.tile([H, 512], f32, tag="P")
                for c in range(NCH):
                    nc.tensor.matmul(
                        vT_ps[:],
                        wv_sb[:, c, :],
                        xT_tiles[c // 2][:, c % 2, sl],
                        start=(c == 0),
                        stop=(c == NCH - 1),
                    )
                nc.vector.tensor_scalar_add(vT_sb[:, sl], vT_ps[:], bv_sb[:])
                emit_warm(512)
                # v tiles of this block into v_aug (PE transpose + copy)
                for t in range(4 * blk, 4 * blk + 4):
                    vtr_ps = pM.tile([128, H], bf16, tag="M")
                    nc.tensor.transpose(
                        vtr_ps[:], vT_sb[:, t * 128:(t + 1) * 128], id_b[0:H, 0:H]
                    )
                    nc.scalar.copy(v_aug[:, t, 0:H], vtr_ps[:])

            def emit_score_pairs(blk, kds):
                qsl = slice(blk * 512, (blk + 1) * 512)
                for kd in kds:   # pairs of k-tiles, row-tiled 2x on the PE
                    ka, kb = 2 * kd, 2 * kd + 1
                    sc_ps = pS.tile([128, 2, 512], f32, tag="S")
                    # row group 0-63: kT/qT live at partitions 0-63
                    nc.tensor.matmul(
                        sc_ps[:, 0, :],
                        kq_sb[0:64, ka * 128:(ka + 1) * 128],
                        qkT_sb[0:64, qsl],
                        start=True,
                        stop=True,
                    )
                    # row group 64-127: kT at qkT rows 64-127, qT at kq 64-127
                    nc.tensor.matmul(
                        sc_ps[:, 1, :],
                        qkT_sb[64:128, kb * 128:(kb + 1) * 128],
                        kq_sb[64:128, qsl],
                        start=True,
                        stop=True,
                    )
                    e_pair = esb.tile([128, 2, 512], bf16, tag="e")
                    nc.scalar.activation(
                        e_pair[:], sc_ps[:], ACT_EXP, bias=0.0, scale=0.125,
                    )
                    nc.vector.tensor_mul(
                        probsT[:, blk, ka:ka + 2, :],
                        e_pair[:],
                        m_bf[:, ka:ka + 2, qsl],
                    )

            pv_ps_tiles = [None] * NB

            def emit_pv_mms(blk, kts):
                if pv_ps_tiles[blk] is None:
                    pv_ps_tiles[blk] = pPV.tile(
                        [H + 1, 512], f32, tag="pv", name=f"pv{blk}"
                    )
                pv_ps = pv_ps_tiles[blk]
                for kt in kts:
                    nc.tensor.matmul(
                        pv_ps[:],
                        v_aug[:, kt, :],
                        probsT[:, blk, kt, :],
                        start=(kt == 0),
                        stop=(kt == NT - 1),
                    )

            def emit_pv_finish(blk):
                oT_sb = sb.tile([H + 1, 512], f32, tag="oT")
                nc.vector.tensor_copy(oT_sb[:], pv_ps_tiles[blk][:])
                oT_tiles[blk] = oT_sb

            def emit_out(blk):
                oT_sb = oT_tiles[blk]
                out_sb = sb.tile([128, 4, H], f32, tag="osb")
                for qq in range(4):
                    o2_ps = pM.tile([128, H + 1], f32, tag="M")
                    nc.tensor.transpose(
                        o2_ps[:],
                        oT_sb[:, qq * 128:(qq + 1) * 128],
                        id_f[0:H + 1, 0:H + 1],
                    )
                    rcp = sb.tile([128, 1], f32, tag="rcp")
                    nc.vector.reciprocal(rcp[:], o2_ps[:, H:H + 1])
                    nc.vector.tensor_scalar_mul(
                        out_sb[:, qq, :], o2_ps[:, 0:H], rcp[:]
                    )
                nc.gpsimd.dma_start(
                    bass.AP(
                        tensor=out_d,
                        offset=blk * 512 * H,
                        ap=[[H, 128], [128 * H, 4], [1, H]],
                    ),
                    out_sb[:],
                )

            # phase 1, hybrid diagonal: after proj(b) emit all pairs
            # (b, kd <= 2b+1) — the exp chain gets 2/4/6/8 pairs per
            # segment instead of starving between projections.
            for b in range(NB):
                emit_proj_qk(b)
                emit_score_pairs(b, list(range(2 * b + 2)))
                emit_proj_v(b)
            # phase 2: leftover pairs with PV chains interleaved; PV(3)
            # drains first (its pairs completed in segment 3), the last
            # block's PV is split so only 4 matmuls trail the final exp.
            emit_score_pairs(0, [2, 3])
            emit_pv_mms(3, list(range(8)))
            emit_warm()
            emit_score_pairs(0, [4, 5])
            emit_pv_mms(3, list(range(8, NT)))
            emit_pv_finish(3)
            emit_warm()
            emit_score_pairs(0, [6, 7])
            emit_out(3)
            emit_pv_mms(0, list(range(8)))
            emit_warm()
            emit_score_pairs(1, [4, 5])
            emit_pv_mms(0, list(range(8, NT)))
            emit_pv_finish(0)
            emit_warm()
            emit_score_pairs(1, [6, 7])
            emit_out(0)
            emit_pv_mms(1, list(range(8)))
            emit_warm()
            emit_pv_mms(2, list(range(12)))
            emit_score_pairs(2, [6, 7])
            emit_pv_mms(1, list(range(8, NT)))
            emit_pv_finish(1)
            emit_out(1)
            emit_pv_mms(2, list(range(12, NT)))
            emit_pv_finish(2)
            emit_out(2)

    nc.compile()
    return nc


_NC_CACHE = None


def _get_nc():
    global _NC_CACHE
    if _NC_CACHE is None:
        _NC_CACHE = build()
    return _NC_CACHE


def _prep_core_inputs(inputs):
    x = np.asarray(inputs["input"], dtype=np.float32)
    m = np.asarray(inputs["mask"])
    W_q = np.asarray(inputs["W_q"], dtype=np.float32)
    W_k = np.asarray(inputs["W_k"], dtype=np.float32)
    W_v = np.asarray(inputs["W_v"], dtype=np.float32)
    wqk = (
        np.concatenate([W_q.T, W_k.T], axis=1)
        .reshape(NCH, 128, 128).transpose(1, 0, 2)
        .reshape(128, NCH * 128).astype(ml_dtypes.bfloat16)
    )
    wv = (
        W_v.T.reshape(NCH, 128, H).transpose(1, 0, 2)
        .reshape(128, NCH * H).astype(ml_dtypes.bfloat16)
    )
    bqk = np.concatenate(
        [np.asarray(inputs["b_q"]), np.asarray(inputs["b_k"])]
    ).astype(np.float32)
    bv = np.asarray(inputs["b_v"], dtype=np.float32)
    shared = {"wqk": wqk, "wv": wv, "bqk": bqk, "bv": bv}
    in_maps = []
    for i in range(B):
        in_maps.append(
            {
                "xT": x[i].T.reshape(NCH, 128, S).transpose(1, 0, 2)
                .reshape(128, NCH * S).astype(ml_dtypes.bfloat16),
                "maskT": m[i].T.reshape(NT, 128, S).transpose(1, 0, 2)
                .reshape(128, NT * S).astype(ml_dtypes.bfloat16),
                **shared,
            }
        )
    return in_maps


def run(inputs, trace=False, trace_cores=None):
    nc = _get_nc()
    in_maps = _prep_core_inputs(inputs)
    res = run_bass_kernel_spmd(
        nc,
        in_maps,
        core_ids=list(range(B)),
        trace=trace,
        trace_cores=trace_cores,
    )
    out = np.stack([res.results[i]["out"] for i in range(B)])
    return out, res


def kernel(**inputs) -> np.ndarray:
    out, _ = run(inputs, trace=False)
    return out


# revision 35
# speedup vs baseline: 1.1103x; 1.1103x over previous
"""Single-head attention (B=8, S=2048, D=1024, H=64) on 8 TRN2 NeuronCores.

Sharding: data-parallel over batch — one batch element per core, Q/K/V
weights replicated. No collectives; host gathers the 8 per-core outputs.

Host-side layout prep (per core): x fed pre-transposed as xT [D, S] bf16,
mask fed pre-transposed as maskT [S, S] bf16 (0.0/1.0), weights fed as
W^T bf16 with W_q/W_k fused into one [D, 128] stationary block.

Per-core pipeline:
  phase 1: qkT [128, S] = (Wqk^T)-stationary matmuls over 8 d-chunks
           (q rows 0-63, k rows 64-127), bias via tensor_scalar_add;
           kq_sb [128, S] = partition-swapped copy (SBUF->SBUF DMA) so
           kT also lives at partitions 0-63 and qT at 64-127;
           vT [64, S] similarly, then PE-transposed into v_aug [S,H+1]
           with a ones column (gives softmax denominators for free).
           Block-0 score pairs are interleaved into this loop so the
           scalar engine's exp chain starts as early as possible.
  phase 2: per 512-wide q-block: scoresT [k,q] computed directly
           (kT stationary, qT moving; K=64 row-tiled 2x: pair of k-tiles
           runs concurrently in the two PE row halves), exp via scalar
           ACT (scale=1/8), mask applied as a bf16 multiply;
           outT[65, q] += v_aug[k-tile].T @ probsT (PSUM accum over k)
           PE-transpose back, multiply by reciprocal of the ones-row,
           DMA out.  PV of block b-1 is emitted between score blocks to
           avoid head-of-line stalls on the PE queue.

Queue assignment: sync = x + mask input streams; gpsimd = weights, kq
partition-swap, output (pure DMA dispatch, no compute); scalar = exp (+
v_aug copies); vector = bias adds, mask multiplies, normalize.
"""

import sys
import types

import numpy as np
import ml_dtypes

import concourse.bass as bass
import concourse.mybir as mybir
import concourse.tile as tile
from concourse import bacc
from concourse.bass_utils import run_bass_kernel_spmd
from concourse.masks import make_identity

B, S, D, H = 8, 2048, 1024, 64
NT = S // 128          # 16 k-tiles of 128
NCH = D // 128         # 8 contraction chunks
NB = 4                 # q-blocks of 512

f32 = mybir.dt.float32
bf16 = mybir.dt.bfloat16
ACT_EXP = mybir.ActivationFunctionType.Exp


def install_ntff_hook():
    """RL-container antenv stub lacks axon_hooks; inject it so trace=True
    under axon can capture NTFF profiles. Harmless if already present."""
    if "antenv.axon_hooks" in sys.modules:
        return
    try:
        mod = types.ModuleType("antenv.axon_hooks")
        state = {"hook": None}
        mod.set_axon_ntff_profile_hook = lambda h: state.__setitem__("hook", h)
        mod.get_axon_ntff_profile_hook = lambda: state["hook"]
        sys.modules["antenv.axon_hooks"] = mod
        import antenv

        antenv.axon_hooks = mod
        from trn_agent_boot.trn_boot import _ntff_profile_via_ctypes

        mod.set_axon_ntff_profile_hook(
            _ntff_profile_via_ctypes("/opt/axon/libaxon_pjrt.so")
        )
    except Exception:
        pass


def build():
    nc = bacc.Bacc("TRN2", target_bir_lowering=False, debug=False, num_devices=8)

    xT_d = nc.dram_tensor("xT", [128, NCH * S], bf16, kind="ExternalInput")
    mT_d = nc.dram_tensor("maskT", [128, NT * S], bf16, kind="ExternalInput")
    wqk_d = nc.dram_tensor("wqk", [128, NCH * 128], bf16, kind="ExternalInput")
    wv_d = nc.dram_tensor("wv", [128, NCH * H], bf16, kind="ExternalInput")
    bqk_d = nc.dram_tensor("bqk", [128], f32, kind="ExternalInput")
    bv_d = nc.dram_tensor("bv", [H], f32, kind="ExternalInput")
    out_d = nc.dram_tensor("out", [S, H], f32, kind="ExternalOutput")

    with tile.TileContext(nc) as tc:
        with (
            tc.tile_pool(name="singles", bufs=1) as singles,
            tc.tile_pool(name="sb", bufs=2) as sb,
            tc.tile_pool(name="esb", bufs=6) as esb,
            tc.tile_pool(name="pP", bufs=2, space="PSUM") as pP,
            tc.tile_pool(name="pS", bufs=2, space="PSUM") as pS,
            tc.tile_pool(name="pM", bufs=1, space="PSUM") as pM,
            tc.tile_pool(name="pPV", bufs=1, space="PSUM") as pPV,
        ):
            # ---- constants -------------------------------------------------
            id_b = singles.tile([128, 128], bf16)
            make_identity(nc, id_b[:])
            id_f = singles.tile([128, 128], f32)
            make_identity(nc, id_f[:])
            id2_sb = singles.tile([128, 512], bf16)
            nc.gpsimd.memset(id2_sb[:], 1.0)

            def emit_warm(n=256):
                wps = pP.tile([128, 512], f32, tag="P", name="warm")
                nc.tensor.matmul(
                    wps[:, 0:n],
                    id_b[:],
                    id2_sb[:, 0:n],
                    start=True,
                    stop=True,
                )

            # HAM pre-burst: saturate the PE with no-dep fillers so the clock
            # gate opens before the first real matmul arrives.
            for _ in range(6):
                emit_warm(512)

            # ---- sync ring: x half 0, weights, x half 1, mask groups -------
            xT_tiles = [
                singles.tile([128, 2, S], bf16, name=f"xT{i}") for i in range(4)
            ]
            wqk_sb = singles.tile([128, NCH, 128], bf16)
            wv_sb = singles.tile([128, NCH, H], bf16)
            bqk_sb = singles.tile([128, 1], f32)
            bv_sb = singles.tile([H, 1], f32)
            m_bf = singles.tile([128, NT, S], bf16)

            def dma_x_q(qq):
                nc.sync.dma_start(
                    xT_tiles[qq][:],
                    bass.AP(
                        tensor=xT_d,
                        offset=qq * 2 * S,
                        ap=[[NCH * S, 128], [1, 2 * S]],
                    ),
                )

            nc.sync.dma_start(
                wqk_sb[:],
                bass.AP(tensor=wqk_d, offset=0, ap=[[NCH * 128, 128], [1, NCH * 128]]),
            )
            nc.sync.dma_start(
                wv_sb[:],
                bass.AP(tensor=wv_d, offset=0, ap=[[NCH * H, 128], [1, NCH * H]]),
            )
            nc.sync.dma_start(
                bqk_sb[:], bass.AP(tensor=bqk_d, offset=0, ap=[[1, 128], [0, 1]])
            )
            nc.sync.dma_start(
                bv_sb[:], bass.AP(tensor=bv_d, offset=0, ap=[[1, H], [0, 1]])
            )
            for qq in range(4):
                dma_x_q(qq)
            for g in range(4):
                nc.sync.dma_start(
                    m_bf[:, g * 4:(g + 1) * 4, :],
                    bass.AP(
                        tensor=mT_d,
                        offset=g * 4 * S,
                        ap=[[NT * S, 128], [1, 4 * S]],
                    ),
                )

            # ---- persistent activations -----------------------------------
            qkT_sb = singles.tile([128, S], bf16)   # q rows 0-63, k rows 64-127
            kq_sb = singles.tile([128, S], bf16)    # k rows 0-63, q rows 64-127
            vT_sb = singles.tile([H, S], bf16)
            v_aug = singles.tile([128, NT, H + 1], bf16)
            nc.gpsimd.memset(v_aug[:, :, H:H + 1], 1.0)

            probsT = singles.tile([128, NB, NT, 512], bf16)
            oT_tiles = [None] * NB

            def emit_proj_qk(blk):
                sl = slice(blk * 512, (blk + 1) * 512)
                qk_ps = pP.tile([128, 512], f32, tag="P")
                for c in range(NCH):
                    nc.tensor.matmul(
                        qk_ps[:],
                        wqk_sb[:, c, :],
                        xT_tiles[c // 2][:, c % 2, sl],
                        start=(c == 0),
                        stop=(c == NCH - 1),
                    )
                nc.vector.tensor_scalar_add(qkT_sb[:, sl], qk_ps[:], bqk_sb[:])
                # partition swap: kT to rows 0-63, qT duplicate to rows 64-127
                nc.gpsimd.dma_start(kq_sb[0:64, sl], qkT_sb[64:128, sl])
                nc.gpsimd.dma_start(kq_sb[64:128, sl], qkT_sb[0:64, sl])

            def emit_proj_v(blk):
                sl = slice(blk * 512, (blk + 1) * 512)
                vT_ps = pP.tile([H, 512], f32, tag="P")
                for c in range(NCH):
                    nc.tensor.matmul(
                        vT_ps[:],
                        wv_sb[:, c, :],
                        xT_tiles[c // 2][:, c % 2, sl],
                        start=(c == 0),
                        stop=(c == NCH - 1),
                    )
                nc.vector.tensor_scalar_add(vT_sb[:, sl], vT_ps[:], bv_sb[:])
                emit_warm(512)
                # v tiles of this block into v_aug (PE transpose + copy)
                for t in range(4 * blk, 4 * blk + 4):
                    vtr_ps = pM.tile([128, H], bf16, tag="M")
                    nc.tensor.transpose(
                        vtr_ps[:], vT_sb[:, t * 128:(t + 1) * 128], id_b[0:H, 0:H]
                    )
                    nc.scalar.copy(v_aug[:, t, 0:H], vtr_ps[:])

            def emit_score_pairs(blk, kds):
                qsl = slice(blk * 512, (blk + 1) * 512)
                for kd in kds:   # pairs of k-tiles, row-tiled 2x on the PE
                    ka, kb = 2 * kd, 2 * kd + 1
                    sc_ps = pS.tile([128, 2, 512], f32, tag="S")
                    # row group 0-63: kT/qT live at partitions 0-63
                    nc.tensor.matmul(
                        sc_ps[:, 0, :],
                        kq_sb[0:64, ka * 128:(ka + 1) * 128],
                        qkT_sb[0:64, qsl],
                        start=True,
                        stop=True,
                    )
                    # row group 64-127: kT at qkT rows 64-127, qT at kq 64-127
                    nc.tensor.matmul(
                        sc_ps[:, 1, :],
                        qkT_sb[64:128, kb * 128:(kb + 1) * 128],
                        kq_sb[64:128, qsl],
                        start=True,
                        stop=True,
                    )
                    e_pair = esb.tile([128, 2, 512], bf16, tag="e")
                    nc.scalar.activation(
                        e_pair[:], sc_ps[:], ACT_EXP, bias=0.0, scale=0.125,
                    )
                    nc.vector.tensor_mul(
                        probsT[:, blk, ka:ka + 2, :],
                        e_pair[:],
                        m_bf[:, ka:ka + 2, qsl],
                    )

            pv_ps_tiles = [None] * NB

            def emit_pv_mms(blk, kts):
                if pv_ps_tiles[blk] is None:
                    pv_ps_tiles[blk] = pPV.tile(
                        [H + 1, 512], f32, tag="pv", name=f"pv{blk}"
                    )
                pv_ps = pv_ps_tiles[blk]
                for kt in kts:
                    nc.tensor.matmul(
                        pv_ps[:],
                        v_aug[:, kt, :],
                        probsT[:, blk, kt, :],
                        start=(kt == 0),
                        stop=(kt == NT - 1),
                    )

            def emit_pv_finish(blk):
                oT_sb = sb.tile([H + 1, 512], f32, tag="oT")
                nc.vector.tensor_copy(oT_sb[:], pv_ps_tiles[blk][:])
                oT_tiles[blk] = oT_sb

            def emit_out(blk):
                oT_sb = oT_tiles[blk]
                out_sb = sb.tile([128, 4, H], f32, tag="osb")
                for qq in range(4):
                    o2_ps = pM.tile([128, H + 1], f32, tag="M")
                    nc.tensor.transpose(
                        o2_ps[:],
                        oT_sb[:, qq * 128:(qq + 1) * 128],
                        id_f[0:H + 1, 0:H + 1],
                    )
                    rcp = sb.tile([128, 1], f32, tag="rcp")
                    nc.vector.reciprocal(rcp[:], o2_ps[:, H:H + 1])
                    nc.vector.tensor_scalar_mul(
                        out_sb[:, qq, :], o2_ps[:, 0:H], rcp[:]
                    )
                nc.gpsimd.dma_start(
                    bass.AP(
                        tensor=out_d,
                        offset=blk * 512 * H,
                        ap=[[H, 128], [128 * H, 4], [1, H]],
                    ),
                    out_sb[:],
                )

            # phase 1 with block-0 scores interleaved right after each qk
            # chain (v work queued behind, off the exp critical path)
            for b in range(NB):
                emit_proj_qk(b)
                emit_score_pairs(0, [2 * b, 2 * b + 1])
                emit_proj_v(b)
            # phase 2: PV(b-1) in two half-chains between score chunks so
            # the scalar exp chain stays fed without LDWEIGHTS thrash.
            for b in range(1, NB):
                for kd in (0, 1, 2, 3):
                    emit_score_pairs(b, [kd])
                    emit_warm()
                emit_pv_mms(b - 1, list(range(8)))
                emit_warm()
                for kd in (4, 5, 6, 7):
                    emit_score_pairs(b, [kd])
                    emit_warm()
                    if b == NB - 1 and kd == 5:
                        emit_pv_mms(NB - 1, list(range(8)))
                emit_pv_mms(b - 1, list(range(8, NT)))
                emit_warm()
                emit_pv_finish(b - 1)
                emit_out(b - 1)
            emit_pv_mms(NB - 1, list(range(8, NT)))
            emit_pv_finish(NB - 1)
            emit_out(NB - 1)

    nc.compile()
    return nc


_NC_CACHE = None


def _get_nc():
    global _NC_CACHE
    if _NC_CACHE is None:
        _NC_CACHE = build()
    return _NC_CACHE


def _prep_core_inputs(inputs):
    x = np.asarray(inputs["input"], dtype=np.float32)
    m = np.asarray(inputs["mask"])
    W_q = np.asarray(inputs["W_q"], dtype=np.float32)
    W_k = np.asarray(inputs["W_k"], dtype=np.float32)
    W_v = np.asarray(inputs["W_v"], dtype=np.float32)
    wqk = (
        np.concatenate([W_q.T, W_k.T], axis=1)
        .reshape(NCH, 128, 128).transpose(1, 0, 2)
        .reshape(128, NCH * 128).astype(ml_dtypes.bfloat16)
    )
    wv = (
        W_v.T.reshape(NCH, 128, H).transpose(1, 0, 2)
        .reshape(128, NCH * H).astype(ml_dtypes.bfloat16)
    )
    bqk = np.concatenate(
        [np.asarray(inputs["b_q"]), np.asarray(inputs["b_k"])]
    ).astype(np.float32)
    bv = np.asarray(inputs["b_v"], dtype=np.float32)
    shared = {"wqk": wqk, "wv": wv, "bqk": bqk, "bv": bv}
    in_maps = []
    for i in range(B):
        in_maps.append(
            {
                "xT": x[i].T.reshape(NCH, 128, S).transpose(1, 0, 2)
                .reshape(128, NCH * S).astype(ml_dtypes.bfloat16),
                "maskT": m[i].T.reshape(NT, 128, S).transpose(1, 0, 2)
                .reshape(128, NT * S).astype(ml_dtypes.bfloat16),
                **shared,
            }
        )
    return in_maps


def run(inputs, trace=False, trace_cores=None):
    nc = _get_nc()
    in_maps = _prep_core_inputs(inputs)
    res = run_bass_kernel_spmd(
        nc,
        in_maps,
        core_ids=list(range(B)),
        trace=trace,
        trace_cores=trace_cores,
    )
    out = np.stack([res.results[i]["out"] for i in range(B)])
    return out, res


def kernel(**inputs) -> np.ndarray:
    out, _ = run(inputs, trace=False)
    return out
